# revision 8
# baseline (speedup 1.0000x reference)
"""CrossModalAttention Trainium2 kernel (8 NeuronCores, SPMD, no collectives).

Reference computation (B=4, S=2048, E=512, H=8, HD=64):
  Q = q_mod @ Wq + bq ; K = k_mod @ Wk + bk ; V = v_mod @ Wv + bv   (per head)
  scores = (Q K^T / sqrt(HD)) * modal_compat[h] ; attn = softmax(scores)
  out = (attn @ V) @ Wo + bo ; LayerNorm(out + q_mod) * gamma + beta

Sharding: core c handles batch b=c//2, query-rows half=c%2 (1024 rows each).
K/V are computed per batch on both cores of a pair (duplicated) so there are
no collectives; each core produces complete output rows.

Layout strategy (all on-chip, feature-major "T" = transposed):
  - host pre-transposes inputs to feature-major; compat/8 folded into Wq,bq
  - projections produce Q.T/K.T feature-major and V token-major (+ones col)
  - scores computed transposed S.T[keys, q] per head (K=64 matmuls, head
    pairs ride different PE row groups), softmax denominator comes from the
    ones column appended to V (M=65 attnV matmuls)
  - exp on ScalarE in head-pair batches [128, 2048]; no max subtraction
    (scores are O(15) at most; fp32 exp is exact enough and overflow-free)
  - normalization deferred: attended.T scaled by 1/denom (DMA partition
    broadcast + reciprocal), then out-proj token-major, +bias via K=1
    ones-matmuls, residual + LayerNorm on VectorE, DMA out.
  - matmuls in float32r (~2^-13 rounding) for 4x PE throughput vs fp32.
"""
import sys
sys.path.insert(0, "/opt/trn_rl_repo")
import numpy as np

B, S, E, H = 4, 2048, 512, 8
HD = E // H
LN_EPS = 1e-5
N_CORES = 8
T = S // 2          # query rows per core
KT = S // 128       # key tiles (16)
TT = T // 128       # out row tiles per core (8)
NPAIR = H // 2      # head pairs (4)

_CACHE = {}


def build_nc(reps: int = 1):
    import concourse.tile as tile
    from concourse import bacc, mybir

    f32 = mybir.dt.float32
    f32r = mybir.dt.float32r
    Exp = mybir.ActivationFunctionType.Exp
    Sqrt = mybir.ActivationFunctionType.Sqrt
    Alu = mybir.AluOpType

    nc = bacc.Bacc("TRN2", target_bir_lowering=False, debug=False,
                   enable_asserts=True, num_devices=N_CORES)
    dram = {}
    for name, shape, dt in [
        ("xqt", (E, T), f32r), ("xkt", (E, S), f32r), ("xvt", (E, S), f32r),
        ("wq", (E, E), f32r), ("wk", (E, E), f32r), ("wv", (E, E), f32r),
        ("wo", (E, E), f32r),
        ("bq", (E,), f32), ("bk", (E,), f32),
        ("bv", (1, E), f32r), ("bo", (1, E), f32r),
        ("xq_res", (T, E), f32), ("gamma", (E,), f32), ("beta", (E,), f32),
    ]:
        dram[name] = nc.dram_tensor(name, shape, dt, kind="ExternalInput").ap()
    out_d = nc.dram_tensor("out", (T, E), f32, kind="ExternalOutput").ap()

    import concourse.bass as bass

    def pbcast(ap, parts):
        """AP view broadcasting partition dim (step 0) to `parts`."""
        return bass.AP(tensor=ap.tensor, offset=ap.offset,
                       ap=[[0, parts]] + list(ap.ap[1:]))

    with tile.TileContext(nc) as tc:
        with tc.tile_pool(name="consts", bufs=1) as consts, \
             tc.tile_pool(name="persist", bufs=1) as persist:
            # weights / biases / constants
            wq_sb = consts.tile([128, 4, E], f32r)
            wk_sb = consts.tile([128, 4, E], f32r)
            wv_sb = consts.tile([128, 4, E], f32r)
            wo_sb = consts.tile([128, 4, E], f32r)
            bq_sb = consts.tile([128, 4], f32)
            bk_sb = consts.tile([128, 4], f32)
            bv_sb = consts.tile([1, E], f32r)
            bo_sb = consts.tile([1, E], f32r)
            ones_sb = consts.tile([1, 128], f32r)
            gamma_b = consts.tile([128, E], f32)
            beta_b = consts.tile([128, E], f32)
            eps_sb = consts.tile([128, 1], f32)

            # attention-persistent tensors
            qt_sb = persist.tile([128, 4, T], f32r)    # Q.T feature-major
            kt_sb = persist.tile([128, 4, S], f32r)    # K.T feature-major
            v_sb = persist.tile([128, KT, H, HD + 1], f32r)  # V tokens + ones
            att_sb = persist.tile([128, 4, T], f32r)   # attended.T normalized
            xq_res_sb = persist.tile([128, TT, E], f32)

            nc.sync.dma_start(wq_sb, dram["wq"].rearrange("(k p) e -> p k e", p=128))
            nc.sync.dma_start(wk_sb, dram["wk"].rearrange("(k p) e -> p k e", p=128))
            nc.sync.dma_start(wv_sb, dram["wv"].rearrange("(k p) e -> p k e", p=128))
            nc.sync.dma_start(wo_sb, dram["wo"].rearrange("(k p) e -> p k e", p=128))
            nc.sync.dma_start(bq_sb, dram["bq"].rearrange("(m p) -> p m", p=128))
            nc.sync.dma_start(bk_sb, dram["bk"].rearrange("(m p) -> p m", p=128))
            nc.sync.dma_start(bv_sb, dram["bv"])
            nc.sync.dma_start(bo_sb, dram["bo"])
            nc.sync.dma_start(gamma_b, pbcast(dram["gamma"][None, :], 128))
            nc.sync.dma_start(beta_b, pbcast(dram["beta"][None, :], 128))
            nc.sync.dma_start(xq_res_sb,
                              dram["xq_res"].rearrange("(t p) e -> p t e", p=128))
            nc.gpsimd.memset(ones_sb[:].bitcast(f32), 1.0)
            nc.gpsimd.memset(eps_sb, LN_EPS)
            nc.gpsimd.memset(v_sb[:, :, :, HD:HD + 1].bitcast(f32), 1.0)

            def body():
                # ---------------- Phase 1: projections ----------------
                with tc.tile_pool(name="xin", bufs=1) as xin, \
                     tc.tile_pool(name="xvg", bufs=2) as xvg, \
                     tc.tile_pool(name="pp", bufs=2, space="PSUM") as pp:
                    xqt_sb = xin.tile([128, 4, T], f32r)
                    nc.sync.dma_start(
                        xqt_sb, dram["xqt"].rearrange("(k p) t -> p k t", p=128))

                    # Q.T [e, t] and K.T [e, t]
                    for m in range(4):
                        q_ps = pp.tile([128, 1024], f32, tag="pp")
                        for k in range(4):
                            for n in range(T // 512):
                                nc.tensor.matmul(
                                    q_ps[:, 512 * n:512 * (n + 1)],
                                    wq_sb[:, k, 128 * m:128 * (m + 1)],
                                    xqt_sb[:, k, 512 * n:512 * (n + 1)],
                                    start=(k == 0), stop=(k == 3))
                        nc.vector.tensor_scalar_add(
                            qt_sb[:, m, :], q_ps[:, 0:T], scalar1=bq_sb[:, m:m + 1])
                    for g in range(4):
                        xk_g = xvg.tile([128, 4, 512], f32r, tag="xkg")
                        nc.sync.dma_start(
                            xk_g,
                            dram["xkt"].rearrange("(k p) t -> p k t", p=128)
                            [:, :, 512 * g:512 * (g + 1)])
                        for m in range(4):
                            k_ps = pp.tile([128, 1024], f32, tag="pp")
                            for k in range(4):
                                nc.tensor.matmul(
                                    k_ps[:, 0:512],
                                    wk_sb[:, k, 128 * m:128 * (m + 1)],
                                    xk_g[:, k, :],
                                    start=(k == 0), stop=(k == 3))
                            nc.vector.tensor_scalar_add(
                                kt_sb[:, m, 512 * g:512 * (g + 1)],
                                k_ps[:, 0:512], scalar1=bk_sb[:, m:m + 1])
                    # V token-major [keys, e] in per-head 65-col groups
                    for tg in range(4):
                        xv_g = xvg.tile([128, 4, 512], f32r, tag="xvg")
                        nc.sync.dma_start(
                            xv_g,
                            dram["xvt"].rearrange("(k p) t -> p k t", p=128)
                            [:, :, 512 * tg:512 * (tg + 1)])
                        for ts in range(4):
                            tt = 4 * tg + ts
                            v_ps = pp.tile([128, 1024], f32, tag="pp")
                            for k in range(4):
                                nc.tensor.matmul(
                                    v_ps[:, 0:512],
                                    xv_g[:, k, 128 * ts:128 * (ts + 1)],
                                    wv_sb[:, k, :], start=(k == 0), stop=False)
                            nc.tensor.matmul(
                                v_ps[:, 0:512], ones_sb, bv_sb,
                                start=False, stop=True)
                            nc.vector.tensor_copy(
                                v_sb[:, tt, :, 0:HD],
                                v_ps[:, 0:512].rearrange("p (h d) -> p h d", h=H))

                # ---------------- Phase 2: attention ----------------
                with tc.tile_pool(name="esc", bufs=3) as escp, \
                     tc.tile_pool(name="dn", bufs=1) as dnp, \
                     tc.tile_pool(name="dnd", bufs=1, space="DRAM") as dndp, \
                     tc.tile_pool(name="ps_s", bufs=1, space="PSUM") as ps_s, \
                     tc.tile_pool(name="ps_at", bufs=1, space="PSUM") as ps_at:
                    for p in range(NPAIR):
                        at0 = ps_at.tile([65, T], f32, tag="at0")
                        at1 = ps_at.tile([65, T], f32, tag="at1")
                        for kt in range(KT):
                            s_ps = ps_s.tile([128, 2048], f32, tag="s")
                            for hi in range(2):
                                pr = 64 * hi
                                lhsT = kt_sb[pr:pr + 64, p,
                                             128 * kt:128 * (kt + 1)]
                                for n in range(2):
                                    c0 = 1024 * hi + 512 * n
                                    nc.tensor.matmul(
                                        s_ps[:, c0:c0 + 512], lhsT,
                                        qt_sb[pr:pr + 64, p,
                                              512 * n:512 * (n + 1)],
                                        start=True, stop=True)
                            esc = escp.tile([128, 2048], f32r, tag="esc")
                            nc.scalar.activation(out=esc, in_=s_ps, func=Exp)
                            for hi, at in ((0, at0), (1, at1)):
                                for n in range(2):
                                    nc.tensor.matmul(
                                        at[:, 512 * n:512 * (n + 1)],
                                        v_sb[:, kt, 2 * p + hi, :],
                                        esc[:, 1024 * hi + 512 * n:
                                            1024 * hi + 512 * (n + 1)],
                                        start=(kt == 0), stop=(kt == KT - 1))
                        # normalize by softmax denominator (row 64 = sums)
                        den = dnp.tile([1, 2048], f32, tag="den")
                        nc.vector.tensor_copy(den[0:1, 0:1024], at0[64:65, :])
                        nc.vector.tensor_copy(den[0:1, 1024:2048], at1[64:65, :])
                        den_d = dndp.tile([1, 2048], f32, tag="dend")
                        nc.sync.dma_start(den_d, den)
                        r2 = dnp.tile([128, 2048], f32, tag="r2")
                        nc.sync.dma_start(r2, pbcast(den_d[0:1, :], 128))
                        nc.vector.reciprocal(r2, r2)
                        nc.vector.tensor_mul(
                            att_sb[0:64, p, :], at0[0:64, :], r2[0:64, 0:1024])
                        nc.vector.tensor_mul(
                            att_sb[64:128, p, :], at1[0:64, :], r2[64:128, 1024:2048])

                # ---------------- Phase 3: out-proj + residual + LN ----------------
                with tc.tile_pool(name="ln", bufs=3) as ln, \
                     tc.tile_pool(name="po", bufs=2, space="PSUM") as po:
                    for tt in range(TT):
                        o_ps = po.tile([128, E], f32, tag="o")
                        for k in range(4):
                            nc.tensor.matmul(
                                o_ps, att_sb[:, k, 128 * tt:128 * (tt + 1)],
                                wo_sb[:, k, :], start=(k == 0), stop=False)
                        nc.tensor.matmul(o_ps, ones_sb, bo_sb,
                                         start=False, stop=True)
                        x_sb = ln.tile([128, E], f32, tag="x")
                        nc.vector.tensor_add(x_sb, o_ps, xq_res_sb[:, tt, :])
                        stats = ln.tile([128, 6], f32, tag="stats")
                        nc.vector.bn_stats(stats, x_sb)
                        mv = ln.tile([128, 2], f32, tag="mv")
                        nc.vector.bn_aggr(mv, stats)
                        sd = ln.tile([128, 1], f32, tag="sd")
                        nc.scalar.activation(out=sd, in_=mv[:, 1:2], func=Sqrt,
                                             bias=eps_sb)
                        rstd = ln.tile([128, 1], f32, tag="rstd")
                        nc.vector.reciprocal(rstd, sd)
                        y_sb = ln.tile([128, E], f32, tag="y")
                        nc.vector.tensor_scalar(
                            out=y_sb, in0=x_sb, scalar1=mv[:, 0:1], scalar2=rstd,
                            op0=Alu.subtract, op1=Alu.mult)
                        nc.vector.tensor_mul(y_sb, y_sb, gamma_b)
                        nc.vector.tensor_add(y_sb, y_sb, beta_b)
                        nc.sync.dma_start(
                            out_d[128 * tt:128 * (tt + 1), :], y_sb)

            if reps == 1:
                body()
            else:
                with tc.For_i(0, reps, 1):
                    body()

    nc.compile()
    return nc


def shard_inputs(query_modality, key_modality, value_modality, Wq, bq, Wk, bk,
                 Wv, bv, Wo, bo, modal_compat, ln_gamma, ln_beta):
    """Host-side prep: fold compat into Wq/bq, pre-transpose, shard per core."""
    f32 = np.float32
    q = np.asarray(query_modality, f32)
    k = np.asarray(key_modality, f32)
    v = np.asarray(value_modality, f32)
    compat = np.asarray(modal_compat, f32).reshape(H)
    scale = np.repeat(compat / np.sqrt(HD), HD).astype(f32)     # [E]
    wq_eff = (np.asarray(Wq, f32) * scale[None, :]).astype(f32)
    bq_eff = (np.asarray(bq, f32) * scale).astype(f32)

    shared = {
        "wq": wq_eff, "wk": np.asarray(Wk, f32), "wv": np.asarray(Wv, f32),
        "wo": np.asarray(Wo, f32),
        "bq": bq_eff, "bk": np.asarray(bk, f32),
        "bv": np.asarray(bv, f32).reshape(1, E),
        "bo": np.asarray(bo, f32).reshape(1, E),
        "gamma": np.asarray(ln_gamma, f32), "beta": np.asarray(ln_beta, f32),
    }
    in_maps = []
    for c in range(N_CORES):
        b, half = c // 2, c % 2
        sl = slice(half * T, (half + 1) * T)
        m = dict(shared)
        m["xqt"] = np.ascontiguousarray(q[b, sl, :].T)
        m["xkt"] = np.ascontiguousarray(k[b].T)
        m["xvt"] = np.ascontiguousarray(v[b].T)
        m["xq_res"] = np.ascontiguousarray(q[b, sl, :])
        in_maps.append(m)
    return in_maps


def kernel(**inputs) -> np.ndarray:
    from concourse.bass_utils import run_bass_kernel_spmd

    if "nc" not in _CACHE:
        _CACHE["nc"] = build_nc(reps=1)
    nc = _CACHE["nc"]
    in_maps = shard_inputs(**inputs)
    res = run_bass_kernel_spmd(nc, in_maps, core_ids=list(range(N_CORES)))
    out = np.empty((B, S, E), np.float32)
    for c in range(N_CORES):
        b, half = c // 2, c % 2
        out[b, half * T:(half + 1) * T, :] = res.results[c]["out"]
    return out
